# revision 41
# baseline (speedup 1.0000x reference)
"""Correlation-cycle (Chamfer) loss kernel for Trainium2, 8 NeuronCores.

reference:  P[b,i,j] = ||x_i||^2 + ||y_j||^2 - 2 x_i.y_j   (x=corr_pred, y=corr_target)
            out = (mean_{b,j} min_i clip(P,0,100) + mean_{b,i} min_j clip(P,0,100)) / B

Sharding: B=4 batches x 2 i-halves -> 8 cores. Each core owns an x-half
(2048 rows) and the full y (4096 rows) of one batch.

Per [128 x 2048] psum group (measured-rate design; TTR/GPSIMD unusable
on this HW, PSUM-reading DVE ops are 1x, bf16 SBUF TT is 2x):

  PE:  psum = -2z + x2_i + y2_j    4 data MMs + 4 rank-2 MMs
       (rank-2 stationary rows 0/1 = x2 chunk / ones, zero elsewhere;
        rank-2 moving rows 0/1 = ones / y2 -- 128-partition operands,
        2-partition ones poison HAM warmup)
  ACT: u = bf16(psum)              ~1966ns, own engine
  DVE: colB = min(colB, u)         bf16 SBUF 2x  (tensor_copy 4x for ic 0)
  DVE: row-min tree of 2x TT-mins down to TW=512; host finishes

Host: colB/rowA hold full-P partial mins; min over lanes/cores,
clip(0,100) (commutes with min), then means.
"""

import numpy as np
import ml_dtypes

import concourse.bass as bass
import concourse.mybir as mybir
import concourse.tile as tile
from concourse import bacc
from concourse.bass_utils import run_bass_kernel_spmd

BF16 = ml_dtypes.bfloat16
F32 = np.float32

B, N, D = 4, 4096, 128
NCORES = 8
NI = N // 2          # per-core i range (half a batch)
NJ = N               # full j range
GW = 2048            # psum group width (4 banks)
MMW = 512            # matmul moving width (1 bank)
BIG = 1.0e38         # accumulator init (min identity; fits bf16)
TW = 512             # row-min tree stop width (host finishes)

AluOp = mybir.AluOpType
ActFn = mybir.ActivationFunctionType


def build(ni=NI, nj=NJ, gw=GW, reps=1, scheme="v3"):
    n_ic = ni // 128
    n_jg = nj // gw

    nc = bacc.Bacc("TRN2", target_bir_lowering=False, debug=False,
                   enable_asserts=False, num_devices=NCORES)
    f32 = mybir.dt.float32
    bf16 = mybir.dt.bfloat16

    xT_d = nc.dram_tensor("xT", [128, ni], bf16, kind="ExternalInput")
    m2yT_d = nc.dram_tensor("m2yT", [128, nj], bf16, kind="ExternalInput")
    r2s_d = nc.dram_tensor("r2s", [128, n_ic * 128], bf16, kind="ExternalInput")
    r2m_d = nc.dram_tensor("r2m", [128, nj], bf16, kind="ExternalInput")
    colB_d = nc.dram_tensor("colB", [128, nj], bf16, kind="ExternalOutput")
    rowA_d = nc.dram_tensor("rowA", [128, n_ic * TW], bf16, kind="ExternalOutput")

    with tile.TileContext(nc) as tc:
        with (
            tc.tile_pool(name="persist", bufs=1) as persist,
            tc.tile_pool(name="psum", bufs=2, space="PSUM") as psum_pool,
            tc.tile_pool(name="u", bufs=4) as upool,
            tc.tile_pool(name="u0", bufs=4) as u0pool,
        ):
            xT = persist.tile([128, ni], bf16, name="xT")
            m2yT = persist.tile([128, nj], bf16, name="m2yT")
            r2s = persist.tile([128, n_ic * 128], bf16, name="r2s")
            r2m = persist.tile([128, nj], bf16, name="r2m")
            colB = persist.tile([128, nj], bf16, name="colB")
            rowacc = persist.tile([128, n_ic * TW], bf16, name="rowacc")
            h = gw // 2
            arow = [persist.tile([128, h], bf16, name=f"ar{ic}")
                    for ic in range(n_ic)]

            # DMA order = first-use order: first-group inputs, then
            # second j-chunks, then xT/r2s tails
            fk = min(512, ni)
            nc.sync.dma_start(out=xT[:, :fk], in_=xT_d[:, :fk])
            nc.sync.dma_start(out=r2s[:, :fk], in_=r2s_d[:, :fk])
            fj = min(gw, nj)
            nc.sync.dma_start(out=m2yT[:, :fj], in_=m2yT_d[:, :fj])
            nc.sync.dma_start(out=r2m[:, :fj], in_=r2m_d[:, :fj])
            for c0 in range(fj, nj, gw):
                nc.sync.dma_start(out=m2yT[:, c0:c0 + gw], in_=m2yT_d[:, c0:c0 + gw])
                nc.sync.dma_start(out=r2m[:, c0:c0 + gw], in_=r2m_d[:, c0:c0 + gw])
            if fk < ni:
                nc.sync.dma_start(out=xT[:, fk:], in_=xT_d[:, fk:])
                nc.sync.dma_start(out=r2s[:, fk:], in_=r2s_d[:, fk:])

            def emit_piecewise_first_group(a):
                # (ic=0, jg=0) in 512-wide pieces: ACT/DVE start right
                # after the first data+bias MM pair instead of after 8 MMs
                psum = psum_pool.tile([128, gw], f32, tag="ps", name="ps")
                ups = []
                for q in range(gw // MMW):
                    j0 = q * MMW
                    qs = slice(q * MMW, (q + 1) * MMW)
                    nc.tensor.matmul(psum[:, qs], xT[:, 0:128],
                                     m2yT[:, j0:j0 + MMW],
                                     start=True, stop=False)
                    nc.tensor.matmul(psum[:, qs], r2s[:, 0:128],
                                     r2m[:, j0:j0 + MMW],
                                     start=False, stop=True)
                    up = u0pool.tile([128, MMW], bf16, tag="u0", name="u0")
                    nc.scalar.activation(up[:, :], psum[:, qs],
                                         ActFn.Identity, bias=0.0, scale=1.0)
                    nc.vector.tensor_copy(colB[:, qs], up[:, :])
                    ups.append(up)
                nc.vector.tensor_tensor(a[:, 0:MMW], ups[0][:, :],
                                        ups[2][:, :], AluOp.min)
                nc.vector.tensor_tensor(a[:, MMW:2 * MMW], ups[1][:, :],
                                        ups[3][:, :], AluOp.min)

            def emit_body():
                for ic in range(n_ic):
                    st = slice(ic * 128, (ic + 1) * 128)
                    us = []
                    for jg in range(n_jg):
                        if ic == 0 and jg == 0:
                            emit_piecewise_first_group(arow[0])
                            continue
                        sl = slice(jg * gw, (jg + 1) * gw)
                        psum = psum_pool.tile([128, gw], f32, tag="ps", name="ps")
                        for q in range(gw // MMW):
                            j0 = jg * gw + q * MMW
                            nc.tensor.matmul(
                                psum[:, q * MMW:(q + 1) * MMW],
                                xT[:, st], m2yT[:, j0:j0 + MMW],
                                start=True, stop=False)
                        for q in range(gw // MMW):
                            j0 = jg * gw + q * MMW
                            nc.tensor.matmul(
                                psum[:, q * MMW:(q + 1) * MMW],
                                r2s[:, st], r2m[:, j0:j0 + MMW],
                                start=False, stop=True)
                        u = upool.tile([128, gw], bf16, tag="u", name="u")
                        nc.scalar.activation(u[:, :], psum[:, :],
                                             ActFn.Identity, bias=0.0,
                                             scale=1.0)
                        if ic == 0:
                            nc.vector.tensor_copy(colB[:, sl], u[:, :])
                        else:
                            nc.vector.tensor_tensor(
                                colB[:, sl], u[:, :], colB[:, sl], AluOp.min)
                        us.append(u)
                    # row-min tree: gw*n_jg -> TW in bf16 TT-mins (2x)
                    # (ic 0: piecewise first group already produced L1 in a)
                    a = arow[ic]
                    if ic != 0:
                        nc.vector.tensor_tensor(
                            a[:, :], us[0][:, :h], us[0][:, h:], AluOp.min)
                    for u in (us if ic == 0 else us[1:]):
                        nc.vector.tensor_tensor(
                            a[:, :], a[:, :], u[:, :h], AluOp.min)
                        nc.vector.tensor_tensor(
                            a[:, :], a[:, :], u[:, h:], AluOp.min)
                    w = h
                    while w > TW * 2:
                        w //= 2
                        nc.vector.tensor_tensor(
                            a[:, :w], a[:, :w], a[:, w:2 * w], AluOp.min)
                    rsl = slice(ic * TW, (ic + 1) * TW)
                    nc.vector.tensor_tensor(
                        rowacc[:, rsl], a[:, :TW], a[:, TW:2 * TW], AluOp.min)
                    if ic % 4 == 3 or ic == n_ic - 1:
                        lo = (ic - ic % 4) * TW
                        osl = slice(lo, (ic + 1) * TW)
                        nc.sync.dma_start(out=rowA_d[:, osl],
                                          in_=rowacc[:, osl])
                for c0 in range(0, nj, gw):
                    nc.sync.dma_start(out=colB_d[:, c0:c0 + gw],
                                      in_=colB[:, c0:c0 + gw])

            if reps > 1:
                with tc.For_i(0, reps, 1,
                              hint_engines=(mybir.EngineType.PE,
                                            mybir.EngineType.DVE,
                                            mybir.EngineType.Activation)):
                    emit_body()
            else:
                emit_body()

    nc.compile()
    return nc


def host_prep(x, y, scheme="v3"):
    """Per-core input maps. Core c: batch c//2, i-half c%2."""
    x = np.ascontiguousarray(np.asarray(x, F32))
    y = np.ascontiguousarray(np.asarray(y, F32))
    x16 = x.astype(BF16)
    y16 = y.astype(BF16)
    m2y16 = (y16.astype(F32) * -2.0).astype(BF16)          # exact in bf16
    x2 = (x16.astype(F32) ** 2).sum(-1)                    # [B, N]
    y2 = (y16.astype(F32) ** 2).sum(-1)
    in_maps = []
    for c in range(NCORES):
        b, hh = divmod(c, 2)
        i0 = hh * NI
        r2s = np.zeros((128, NI), BF16)
        r2s[0, :] = x2[b, i0:i0 + NI].astype(BF16)
        r2s[1, :] = 1.0
        r2m = np.zeros((128, N), BF16)
        r2m[0, :] = 1.0
        r2m[1, :] = y2[b].astype(BF16)
        m = {
            "xT": np.ascontiguousarray(x16[b, i0:i0 + NI, :].T),
            "m2yT": np.ascontiguousarray(m2y16[b].T),
            "r2s": r2s,
            "r2m": r2m,
        }
        in_maps.append(m)
    return in_maps, x2, y2


def combine(results, x2, y2, scheme="v3"):
    col_mins = np.empty((B, N), F32)
    row_mins = np.empty((B, N), F32)
    for b in range(B):
        cores = [results[2 * b], results[2 * b + 1]]
        col = np.minimum.reduce([r["colB"].astype(F32).min(0) for r in cores])
        col_mins[b] = np.clip(col, 0.0, 100.0)
        for hh, r in enumerate(cores):
            ra = r["rowA"].astype(F32)                 # [128, n_ic*TW]
            row = ra.reshape(128, NI // 128, TW).min(axis=2).T.reshape(-1)
            i0 = hh * NI
            row_mins[b, i0:i0 + NI] = np.clip(row, 0.0, 100.0)
    out = (col_mins.mean(dtype=np.float64) + row_mins.mean(dtype=np.float64)) / B
    return np.asarray(out, dtype=F32)


_CACHE = {}
TRACE = False
LAST_RESULTS = None
SCHEME = "v3"


def kernel(corr_pred, corr_target):
    global LAST_RESULTS
    key = ("nc", SCHEME)
    if key not in _CACHE:
        _CACHE[key] = build(scheme=SCHEME)
    nc = _CACHE[key]
    in_maps, x2, y2 = host_prep(corr_pred, corr_target, scheme=SCHEME)
    res = run_bass_kernel_spmd(nc, in_maps, core_ids=list(range(NCORES)),
                               trace=TRACE)
    LAST_RESULTS = res
    return combine(res.results, x2, y2, scheme=SCHEME)


# revision 42
# speedup vs baseline: 1.0566x; 1.0566x over previous
"""Correlation-cycle (Chamfer) loss kernel for Trainium2, 8 NeuronCores.

reference:  P[b,i,j] = ||x_i||^2 + ||y_j||^2 - 2 x_i.y_j   (x=corr_pred, y=corr_target)
            out = (mean_{b,j} min_i clip(P,0,100) + mean_{b,i} min_j clip(P,0,100)) / B

Sharding: B=4 batches x 2 i-halves -> 8 cores. Each core owns an x-half
(2048 rows) and the full y (4096 rows) of one batch.

Per [128 x 2048] psum group (measured-rate design; TTR/GPSIMD unusable
on this HW, PSUM-reading DVE ops are 1x, bf16 SBUF TT is 2x):

  PE:  psum = -2z + x2_i + y2_j    4 data MMs + 4 rank-2 MMs
       (rank-2 stationary rows 0/1 = x2 chunk / ones, zero elsewhere;
        rank-2 moving rows 0/1 = ones / y2 -- 128-partition operands,
        2-partition ones poison HAM warmup)
  ACT: u = bf16(psum)              ~1966ns, own engine
  DVE: colB = min(colB, u)         bf16 SBUF 2x  (tensor_copy 4x for ic 0)
  DVE: row-min tree of 2x TT-mins down to TW=512; host finishes

Host: colB/rowA hold full-P partial mins; min over lanes/cores,
clip(0,100) (commutes with min), then means.
"""

import numpy as np
import ml_dtypes

import concourse.bass as bass
import concourse.mybir as mybir
import concourse.tile as tile
from concourse import bacc
from concourse.bass_utils import run_bass_kernel_spmd

BF16 = ml_dtypes.bfloat16
F32 = np.float32

B, N, D = 4, 4096, 128
NCORES = 8
NI = N // 2          # per-core i range (half a batch)
NJ = N               # full j range
GW = 2048            # psum group width (4 banks)
MMW = 512            # matmul moving width (1 bank)
BIG = 1.0e38         # accumulator init (min identity; fits bf16)
TW = 512             # row-min tree stop width (host finishes)

AluOp = mybir.AluOpType
ActFn = mybir.ActivationFunctionType


def build(ni=NI, nj=NJ, gw=GW, reps=1, scheme="v3"):
    n_ic = ni // 128
    n_jg = nj // gw

    nc = bacc.Bacc("TRN2", target_bir_lowering=False, debug=False,
                   enable_asserts=False, num_devices=NCORES)
    f32 = mybir.dt.float32
    bf16 = mybir.dt.bfloat16

    xT_d = nc.dram_tensor("xT", [128, ni], bf16, kind="ExternalInput")
    m2yT_d = nc.dram_tensor("m2yT", [128, nj], bf16, kind="ExternalInput")
    r2s_d = nc.dram_tensor("r2s", [128, n_ic * 128], bf16, kind="ExternalInput")
    r2m_d = nc.dram_tensor("r2m", [128, nj], bf16, kind="ExternalInput")
    colB_d = nc.dram_tensor("colB", [128, nj], bf16, kind="ExternalOutput")
    rowA_d = nc.dram_tensor("rowA", [128, n_ic * TW], bf16, kind="ExternalOutput")

    with tile.TileContext(nc) as tc:
        with (
            tc.tile_pool(name="persist", bufs=1) as persist,
            tc.tile_pool(name="psum", bufs=2, space="PSUM") as psum_pool,
            tc.tile_pool(name="u", bufs=4) as upool,
            tc.tile_pool(name="u0", bufs=4) as u0pool,
        ):
            xT = persist.tile([128, ni], bf16, name="xT")
            m2yT = persist.tile([128, nj], bf16, name="m2yT")
            r2s = persist.tile([128, n_ic * 128], bf16, name="r2s")
            r2m = persist.tile([128, nj], bf16, name="r2m")
            colB = persist.tile([128, nj], bf16, name="colB")
            rowacc = persist.tile([128, n_ic * TW], bf16, name="rowacc")
            h = gw // 2
            arow = [persist.tile([128, h], bf16, name=f"ar{ic}")
                    for ic in range(n_ic)]

            # DMA order = first-use order: first-group inputs, then
            # second j-chunks, then xT/r2s tails
            fk = min(512, ni)
            nc.sync.dma_start(out=xT[:, :fk], in_=xT_d[:, :fk])
            nc.sync.dma_start(out=r2s[:, :fk], in_=r2s_d[:, :fk])
            fj = min(gw, nj)
            nc.sync.dma_start(out=m2yT[:, :fj], in_=m2yT_d[:, :fj])
            nc.sync.dma_start(out=r2m[:, :fj], in_=r2m_d[:, :fj])
            for c0 in range(fj, nj, gw):
                nc.sync.dma_start(out=m2yT[:, c0:c0 + gw], in_=m2yT_d[:, c0:c0 + gw])
                nc.sync.dma_start(out=r2m[:, c0:c0 + gw], in_=r2m_d[:, c0:c0 + gw])
            if fk < ni:
                nc.sync.dma_start(out=xT[:, fk:], in_=xT_d[:, fk:])
                nc.sync.dma_start(out=r2s[:, fk:], in_=r2s_d[:, fk:])

            def emit_piecewise_first_group(a):
                # (ic=0, jg=0) in 512-wide pieces: ACT/DVE start right
                # after the first data+bias MM pair instead of after 8 MMs
                psum = psum_pool.tile([128, gw], f32, tag="ps", name="ps")
                ups = []
                for q in range(gw // MMW):
                    j0 = q * MMW
                    qs = slice(q * MMW, (q + 1) * MMW)
                    nc.tensor.matmul(psum[:, qs], xT[:, 0:128],
                                     m2yT[:, j0:j0 + MMW],
                                     start=True, stop=False)
                    nc.tensor.matmul(psum[:, qs], r2s[:, 0:128],
                                     r2m[:, j0:j0 + MMW],
                                     start=False, stop=True)
                    up = u0pool.tile([128, MMW], bf16, tag="u0", name="u0")
                    nc.scalar.activation(up[:, :], psum[:, qs],
                                         ActFn.Identity, bias=0.0, scale=1.0)
                    nc.vector.tensor_copy(colB[:, qs], up[:, :])
                    ups.append(up)
                nc.vector.tensor_tensor(a[:, 0:MMW], ups[0][:, :],
                                        ups[2][:, :], AluOp.min)
                nc.vector.tensor_tensor(a[:, MMW:2 * MMW], ups[1][:, :],
                                        ups[3][:, :], AluOp.min)

            def emit_body():
                for ic in range(n_ic):
                    st = slice(ic * 128, (ic + 1) * 128)
                    us = []
                    for jg in range(n_jg):
                        sl = slice(jg * gw, (jg + 1) * gw)
                        psum = psum_pool.tile([128, gw], f32, tag="ps", name="ps")
                        for q in range(gw // MMW):
                            j0 = jg * gw + q * MMW
                            nc.tensor.matmul(
                                psum[:, q * MMW:(q + 1) * MMW],
                                xT[:, st], m2yT[:, j0:j0 + MMW],
                                start=True, stop=False)
                        for q in range(gw // MMW):
                            j0 = jg * gw + q * MMW
                            nc.tensor.matmul(
                                psum[:, q * MMW:(q + 1) * MMW],
                                r2s[:, st], r2m[:, j0:j0 + MMW],
                                start=False, stop=True)
                        u = upool.tile([128, gw], bf16, tag="u", name="u")
                        nc.scalar.activation(u[:, :], psum[:, :],
                                             ActFn.Identity, bias=0.0,
                                             scale=1.0)
                        if ic == 0:
                            nc.vector.tensor_copy(colB[:, sl], u[:, :])
                        else:
                            nc.vector.tensor_tensor(
                                colB[:, sl], u[:, :], colB[:, sl], AluOp.min)
                        us.append(u)
                    # row-min tree: gw*n_jg -> TW in bf16 TT-mins (2x)
                    a = arow[ic]
                    nc.vector.tensor_tensor(
                        a[:, :], us[0][:, :h], us[0][:, h:], AluOp.min)
                    for u in us[1:]:
                        nc.vector.tensor_tensor(
                            a[:, :], a[:, :], u[:, :h], AluOp.min)
                        nc.vector.tensor_tensor(
                            a[:, :], a[:, :], u[:, h:], AluOp.min)
                    w = h
                    while w > TW * 2:
                        w //= 2
                        nc.vector.tensor_tensor(
                            a[:, :w], a[:, :w], a[:, w:2 * w], AluOp.min)
                    rsl = slice(ic * TW, (ic + 1) * TW)
                    nc.vector.tensor_tensor(
                        rowacc[:, rsl], a[:, :TW], a[:, TW:2 * TW], AluOp.min)
                    nc.sync.dma_start(out=rowA_d[:, rsl],
                                      in_=rowacc[:, rsl])
                for c0 in range(0, nj, gw):
                    nc.sync.dma_start(out=colB_d[:, c0:c0 + gw],
                                      in_=colB[:, c0:c0 + gw])

            if reps > 1:
                with tc.For_i(0, reps, 1,
                              hint_engines=(mybir.EngineType.PE,
                                            mybir.EngineType.DVE,
                                            mybir.EngineType.Activation)):
                    emit_body()
            else:
                emit_body()

    nc.compile()
    return nc


def host_prep(x, y, scheme="v3"):
    """Per-core input maps. Core c: batch c//2, i-half c%2."""
    x = np.ascontiguousarray(np.asarray(x, F32))
    y = np.ascontiguousarray(np.asarray(y, F32))
    x16 = x.astype(BF16)
    y16 = y.astype(BF16)
    m2y16 = (y16.astype(F32) * -2.0).astype(BF16)          # exact in bf16
    x2 = (x16.astype(F32) ** 2).sum(-1)                    # [B, N]
    y2 = (y16.astype(F32) ** 2).sum(-1)
    in_maps = []
    for c in range(NCORES):
        b, hh = divmod(c, 2)
        i0 = hh * NI
        r2s = np.zeros((128, NI), BF16)
        r2s[0, :] = x2[b, i0:i0 + NI].astype(BF16)
        r2s[1, :] = 1.0
        r2m = np.zeros((128, N), BF16)
        r2m[0, :] = 1.0
        r2m[1, :] = y2[b].astype(BF16)
        m = {
            "xT": np.ascontiguousarray(x16[b, i0:i0 + NI, :].T),
            "m2yT": np.ascontiguousarray(m2y16[b].T),
            "r2s": r2s,
            "r2m": r2m,
        }
        in_maps.append(m)
    return in_maps, x2, y2


def combine(results, x2, y2, scheme="v3"):
    col_mins = np.empty((B, N), F32)
    row_mins = np.empty((B, N), F32)
    for b in range(B):
        cores = [results[2 * b], results[2 * b + 1]]
        col = np.minimum.reduce([r["colB"].astype(F32).min(0) for r in cores])
        col_mins[b] = np.clip(col, 0.0, 100.0)
        for hh, r in enumerate(cores):
            ra = r["rowA"].astype(F32)                 # [128, n_ic*TW]
            row = ra.reshape(128, NI // 128, TW).min(axis=2).T.reshape(-1)
            i0 = hh * NI
            row_mins[b, i0:i0 + NI] = np.clip(row, 0.0, 100.0)
    out = (col_mins.mean(dtype=np.float64) + row_mins.mean(dtype=np.float64)) / B
    return np.asarray(out, dtype=F32)


_CACHE = {}
TRACE = False
LAST_RESULTS = None
SCHEME = "v3"


def kernel(corr_pred, corr_target):
    global LAST_RESULTS
    key = ("nc", SCHEME)
    if key not in _CACHE:
        _CACHE[key] = build(scheme=SCHEME)
    nc = _CACHE[key]
    in_maps, x2, y2 = host_prep(corr_pred, corr_target, scheme=SCHEME)
    res = run_bass_kernel_spmd(nc, in_maps, core_ids=list(range(NCORES)),
                               trace=TRACE)
    LAST_RESULTS = res
    return combine(res.results, x2, y2, scheme=SCHEME)
